# revision 17
# baseline (speedup 1.0000x reference)
"""Trainium2 Bass kernel for nn_EnhancedGNN (3-block GCN + BN + MLP head).

Strategy (8 NeuronCores, node-partitioned graph parallel):
  - Nodes are partitioned contiguously across the 8 cores (12500 each,
    padded to 12544 = 98*128). Each core owns all edges whose dst lies in
    its partition (plus the self-loops of its own nodes).
  - Per GCN layer, each core holds the full feature table
    H~ = dinv * (h @ W) (replicated via AllGather, [100352, 128] f32 in
    DRAM). Messages are fetched with dma_gather (random 512B rows).
  - Segment-sum by dst is a one-hot matmul: for each chunk of 128
    dst-sorted messages, S[msg, dstslot] = (dstrel[msg] == iota[slot])
    built on DVE, then PSUM += S^T @ msgs on TensorE. dinv[dst] is applied
    on PSUM evacuation (ScalarE per-partition scale); dinv[src] is folded
    into the table rows; GCN bias is absorbed by the following BatchNorm.
  - BN statistics are computed with ones-matmuls and a tiny AllReduce.
"""

import numpy as np

import concourse.bacc as bacc
import concourse.mybir as mybir
import concourse.tile as tile
from concourse.bass import AP
from concourse.bass_utils import run_bass_kernel_spmd
from concourse.masks import make_identity

F32 = mybir.dt.float32
F16 = mybir.dt.float16
I16 = mybir.dt.int16
I32 = mybir.dt.int32

N_NODES = 100000
HID = 128
NCORES = 8
EPS = 1e-5
RG = [list(range(NCORES))]
WINB = 4                         # windows per PSUM batch ([128, 512] tile)
KMAX = 16                        # chunks per dma_gather call
NQ = 4                           # SWDGE queues for parallel gather desc-gen


def _set_config(n_nodes=100000, group=32768):
    """Derive all sharding constants (module globals) from the node count."""
    global N_NODES, NSH, TPC, NPAD, TROWS, GROUP, GBASES, GSIZES, NG
    N_NODES = n_nodes
    NSH = N_NODES // NCORES
    TPC = (NSH + 127) // 128
    NPAD = TPC * 128
    TROWS = NCORES * NPAD
    GROUP = group
    GBASES = list(range(0, TROWS, GROUP))
    GSIZES = [min(GROUP, TROWS - b) for b in GBASES]
    NG = len(GBASES)


_set_config()


# ----------------------------------------------------------------------------
# Host-side preprocessing: edge bucketing + per-core streams
# ----------------------------------------------------------------------------

def _preprocess(edge_index, x):
    src = edge_index[0].astype(np.int64)
    dst = edge_index[1].astype(np.int64)
    loop = np.arange(N_NODES, dtype=np.int64)
    src = np.concatenate([src, loop])
    dst = np.concatenate([dst, loop])

    deg = np.bincount(dst, minlength=N_NODES).astype(np.float32)
    dinv = (1.0 / np.sqrt(deg)).astype(np.float32)

    srow = (src // NSH) * NPAD + (src % NSH)     # table row id
    core = dst // NSH
    dloc = dst % NSH
    win = dloc // 128
    drel = dloc % 128
    grp = srow // GROUP

    counts = np.zeros((NCORES, TPC, NG), np.int64)
    percore = []
    for c in range(NCORES):
        m = core == c
        s_c, w_c, g_c, d_c = srow[m], win[m], grp[m], drel[m]
        o = np.lexsort((d_c, g_c, w_c))
        s_c, w_c, g_c, d_c = s_c[o], w_c[o], g_c[o], d_c[o]
        cnt = np.zeros((TPC, NG), np.int64)
        np.add.at(cnt, (w_c, g_c), 1)
        counts[c] = cnt
        percore.append((s_c, d_c, cnt))
    cmax = counts.max(0)
    chunks = -(-cmax // 128)                     # [TPC, NG] ceil
    assert (chunks.sum(1) > 0).all()

    # ---- static call plan shared by every core
    win_total = chunks.sum(1)
    batches = []
    bucket_gids = {}                             # (w, g) -> [chunk gids]
    gid = 0
    nb = -(-TPC // WINB)
    for b in range(nb):
        wins = list(range(b * WINB, min((b + 1) * WINB, TPC)))
        emitted = {w: 0 for w in wins}
        calls = []
        chunk0 = gid
        for g in range(NG):
            pend = []
            for w in wins:
                if chunks[w, g]:
                    bucket_gids.setdefault((w, g), [])
                    pend.extend([w] * chunks[w, g])
            for i0 in range(0, len(pend), KMAX):
                sub = pend[i0:i0 + KMAX]
                ch = []
                for j, w in enumerate(sub):
                    st = emitted[w] == 0
                    emitted[w] += 1
                    sp = emitted[w] == win_total[w]
                    ch.append((w, st, sp))
                    bucket_gids[(w, g)].append(gid + j)
                calls.append(dict(g=g, k=len(sub), chunks=ch, gid0=gid))
                gid += len(sub)
        batches.append(dict(wins=wins, calls=calls, chunk0=chunk0,
                            nchunks=gid - chunk0))
    tot_chunks = gid

    # ---- per-core streams
    streams = []
    for c in range(NCORES):
        s_c, d_c, cnt = percore[c]
        # offsets of each (w, g) bucket in the lexsorted arrays
        flat = cnt.reshape(-1)
        offs = np.concatenate([[0], np.cumsum(flat)[:-1]]).reshape(TPC, NG)
        idx_stream = np.zeros((tot_chunks, 128), np.int16)
        drel_stream = np.full((tot_chunks, 128), -1.0, np.float32)
        for (w, g), gids in bucket_gids.items():
            n = cnt[w, g]
            ncap = len(gids) * 128
            o0 = offs[w, g]
            vals = np.zeros(ncap, np.int64)
            vals[:n] = s_c[o0:o0 + n] - GBASES[g]
            dr = np.full(ncap, -1.0, np.float32)
            dr[:n] = d_c[o0:o0 + n]
            idx_stream[gids] = vals.reshape(-1, 128).astype(np.int16)
            drel_stream[gids] = dr.reshape(-1, 128)
        # wrap indices: chunk j, msg p -> [p % 16, j*8 + p//16], replicate x8
        idx16 = idx_stream.reshape(tot_chunks * 8, 16).T          # [16, 8*T]
        idx_wrapped = np.tile(idx16, (8, 1)).copy()               # [128, 8*T]
        drel_cols = np.ascontiguousarray(drel_stream.T).astype(np.float16)
        dv = np.zeros(NPAD, np.float32)
        dv[:NSH] = dinv[c * NSH:(c + 1) * NSH]
        dinv_cols = np.ascontiguousarray(dv.reshape(TPC, 128).T)  # [128, TPC]
        mk = np.zeros(NPAD, np.float32)
        mk[:NSH] = 1.0
        msk_cols = np.ascontiguousarray(mk.reshape(TPC, 128).T)
        xp = np.zeros((NPAD, 2), np.float32)
        xp[:NSH] = x[c * NSH:(c + 1) * NSH]
        xT = np.ascontiguousarray(xp.T)                           # [2, NPAD]
        streams.append(dict(idxs=idx_wrapped, drel=drel_cols,
                            dinv=dinv_cols, msk=msk_cols, xT=xT))

    plan = dict(batches=batches, tot_chunks=tot_chunks,
                tot_cols=tot_chunks * 8)
    return plan, streams


# ----------------------------------------------------------------------------
# Device program
# ----------------------------------------------------------------------------

def _build_program(plan):
    nc = bacc.Bacc("TRN2", target_bir_lowering=False, debug=False,
                   enable_asserts=True, num_devices=NCORES,
                   num_swdge_queues=NQ)

    def din(name, shape, dt=F32):
        return nc.dram_tensor(name, list(shape), dt, kind="ExternalInput").ap()

    t_idx = din("idxs", [128, plan["tot_cols"]], I16)
    t_drel = din("drel", [128, plan["tot_chunks"]], F16)
    t_dinv = din("dinv", [128, TPC])
    t_msk = din("msk", [128, TPC])
    t_xT = din("xT", [2, NPAD])
    t_We = din("We", [2, HID])
    t_W = {1: din("W1", [HID, HID]), 2: din("W2", [HID, HID]),
           3: din("W3", [HID, HID])}
    t_Wf1 = din("Wf1", [HID, 32])
    t_Wf2 = din("Wf2", [32, 2])
    t_bebc = din("be_bc", [128, HID])
    t_g = {1: din("g1", [1, HID]), 2: din("g2", [1, HID]), 3: din("g3", [1, HID])}
    t_bt = {1: din("bt1", [1, HID]), 2: din("bt2", [1, HID]),
            3: din("bt3", [1, HID])}
    t_gf = din("gf", [1, 32])
    t_btf = din("btf", [1, 32])
    t_bf2 = din("bf2c", [2, 1])
    t_out = nc.dram_tensor("out", [2, NPAD], F32, kind="ExternalOutput").ap()

    from contextlib import ExitStack
    with tile.TileContext(nc) as tc, ExitStack() as st:
        cst = st.enter_context(tc.tile_pool(name="cst", bufs=1))
        sb = st.enter_context(tc.tile_pool(name="sb", bufs=2))
        scp = st.enter_context(tc.tile_pool(name="scp", bufs=4))
        msgp = st.enter_context(tc.tile_pool(name="msgp", bufs=10))
        ps_agg = st.enter_context(tc.tile_pool(name="ps_agg", bufs=1, space="PSUM"))
        ps_st = st.enter_context(tc.tile_pool(name="ps_st", bufs=1, space="PSUM"))
        ps_a = st.enter_context(tc.tile_pool(name="ps_a", bufs=1, space="PSUM"))
        ps_b = st.enter_context(tc.tile_pool(name="ps_b", bufs=1, space="PSUM"))
        dr = st.enter_context(tc.tile_pool(name="dr", bufs=1, space="DRAM"))
        _emit(nc, tc, plan, locals())
    nc.compile()
    return nc


def _emit(nc, tc, plan, pools):
    cst, sb, msgp = pools["cst"], pools["sb"], pools["msgp"]
    scp = pools["scp"]
    ps_agg, ps_st = pools["ps_agg"], pools["ps_st"]
    ps_a, ps_b, dr = pools["ps_a"], pools["ps_b"], pools["dr"]
    t_idx, t_drel = pools["t_idx"], pools["t_drel"]
    t_dinv, t_xT, t_We = pools["t_dinv"], pools["t_xT"], pools["t_We"]
    t_W, t_Wf1, t_Wf2 = pools["t_W"], pools["t_Wf1"], pools["t_Wf2"]
    t_bebc, t_g, t_bt = pools["t_bebc"], pools["t_g"], pools["t_bt"]
    t_gf, t_btf, t_bf2 = pools["t_gf"], pools["t_btf"], pools["t_bf2"]
    t_out = pools["t_out"]
    AO, AF = mybir.AluOpType, mybir.ActivationFunctionType

    # ---- constants
    iota_i = cst.tile([128, 128], I32)
    nc.gpsimd.iota(iota_i[:], pattern=[[1, 128]], base=0, channel_multiplier=0)
    iota_f = cst.tile([128, 128], F16)
    nc.vector.tensor_copy(iota_f[:], iota_i[:])
    ident = cst.tile([128, 128], F32)
    make_identity(nc, ident[:])
    ones_col = cst.tile([128, 1], F32)
    nc.vector.memset(ones_col[:], 1.0)
    ones_row = cst.tile([1, 128], F32)
    nc.vector.memset(ones_row[:], 1.0)
    eps_sb = cst.tile([1, 1], F32)
    nc.vector.memset(eps_sb[:], EPS)

    def load_const(t, shape, dt=F32):
        tl = cst.tile(shape, dt, name=f"c_{t.tensor.name}")
        nc.sync.dma_start(tl[:], t[:])
        return tl

    dinv_sb = load_const(t_dinv, [128, TPC])
    msk_sb = load_const(pools["t_msk"], [128, TPC])
    We_sb = load_const(t_We, [2, HID])
    W_sb = {i: load_const(t_W[i], [HID, HID]) for i in (1, 2, 3)}
    Wf1_sb = load_const(t_Wf1, [HID, 32])
    Wf2_sb = load_const(t_Wf2, [32, 2])
    bebc_sb = load_const(t_bebc, [128, HID])
    g_sb = {i: load_const(t_g[i], [1, HID]) for i in (1, 2, 3)}
    bt_sb = {i: load_const(t_bt[i], [1, HID]) for i in (1, 2, 3)}
    gf_sb = load_const(t_gf, [1, 32])
    btf_sb = load_const(t_btf, [1, 32])
    bf2_sb = load_const(t_bf2, [2, 1], F32)

    h_big = cst.tile([128, NPAD], F32)       # node-major h (residual carrier)
    agg_big = cst.tile([128, NPAD], F32)     # node-major aggregation output
    fpre_big = cst.tile([128, TPC * 32], F32)

    tshard = dr.tile([NPAD, HID], F16, name="tshard")
    tables = {i: dr.tile([TROWS, HID], F16, addr_space="Shared",
                         name=f"table{i}") for i in (1, 2, 3)}
    ar_in = dr.tile([1, 256], F32, name="ar_in")
    ar_outs = {i: dr.tile([1, 256], F32, addr_space="Shared",
                          name=f"ar_out{i}") for i in (1, 2, 3)}
    ar_in_f = dr.tile([1, 64], F32, name="ar_in_f")
    ar_out_f = dr.tile([1, 64], F32, addr_space="Shared", name="ar_out_f")

    def ts(t):
        return slice(t * 128, (t + 1) * 128)

    stage = plan.get("stage", 99)

    def dbg_out(src):
        w = min(NPAD, src.shape[1])
        nc.sync.dma_start(t_out[:, :w], src[:2, :w])

    def emit_table(Wnext_sb, table):
        """tshard <- dinv * (h @ Wnext), then AllGather into table."""
        for t in range(TPC):
            trp = ps_a.tile([128, 128], F32, tag="trp", name="trp")
            nc.tensor.transpose(out=trp[:], in_=h_big[:, ts(t)], identity=ident[:])
            ht = sb.tile([128, 128], F32, tag="ht", name="ht")
            nc.vector.tensor_copy(ht[:], trp[:])
            mmp = ps_b.tile([128, 128], F32, tag="mmp", name="mmp")
            nc.tensor.matmul(out=mmp[:], lhsT=ht[:], rhs=Wnext_sb[:],
                             start=True, stop=True)
            tsh = sb.tile([128, 128], F16, tag="tsh", name="tsh")
            nc.scalar.activation(tsh[:], mmp[:], AF.Copy,
                                 scale=dinv_sb[:, t:t + 1])
            nc.sync.dma_start(tshard[ts(t), :], tsh[:])
        nc.gpsimd.collective_compute(
            "AllGather", mybir.AluOpType.bypass, replica_groups=RG,
            ins=[tshard.opt()], outs=[table.opt()])

    # ------------------------------------------------------------------
    # embed: h = relu(x @ We + be)
    # ------------------------------------------------------------------
    if plan.get("skip_embed"):
        nc.vector.memset(h_big[:], 0.25)
    else:
        for t in range(TPC):
            xt = sb.tile([2, 128], F32, tag="xt", name="xt")
            nc.sync.dma_start(xt[:], t_xT[:, ts(t)])
            mmp = ps_b.tile([128, 128], F32, tag="mmp", name="mmp_e")
            nc.tensor.matmul(out=mmp[:], lhsT=xt[:], rhs=We_sb[:],
                             start=True, stop=True)
            nc.vector.tensor_tensor(out=h_big[:, ts(t)], in0=mmp[:],
                                    in1=bebc_sb[:], op=AO.add)
            nc.vector.tensor_scalar_max(h_big[:, ts(t)], h_big[:, ts(t)], 0.0)
    if stage <= 1:
        dbg_out(h_big)
        return
    if plan.get("simple_table"):
        for t in range(TPC):
            tsh = sb.tile([128, 128], F16, tag="tsh", name="tsh_s")
            nc.vector.tensor_copy(tsh[:], h_big[:, ts(t)])
            nc.sync.dma_start(tshard[ts(t), :], tsh[:])
        nc.gpsimd.collective_compute(
            "AllGather", mybir.AluOpType.bypass, replica_groups=RG,
            ins=[tshard.opt()], outs=[tables[1].opt()])
    else:
        emit_table(W_sb[1], tables[1])
    if stage <= 2:
        tb = sb.tile([2, 128], F32, tag="dbg", name="dbgt")
        nc.sync.dma_start(tb[:], tables[1][:2, :])
        zz = sb.tile([2, NPAD], F32, tag="dbg2", name="dbgz")
        nc.vector.memset(zz[:], 0.0)
        nc.vector.tensor_copy(zz[:, :128], tb[:])
        dbg_out(zz)
        return

    # ------------------------------------------------------------------
    # 3 GCN blocks
    # ------------------------------------------------------------------
    parts = plan.get("parts", ("iseq", "mm", "evac", "stats"))
    dbg_acc = None
    if "mm" not in parts:
        dbg_acc = cst.tile([128, 128], F32)
        nc.vector.memset(dbg_acc[:], 0.0)
    qi = 0
    for layer in (1, 2, 3):
        sum_ps = ps_st.tile([1, 128], F32, tag="sum", name=f"sum{layer}")
        sq_ps = ps_st.tile([1, 128], F32, tag="sq", name=f"sq{layer}")
        for batch in plan["batches"]:
            nch = batch["nchunks"]
            c0 = batch["chunk0"]
            idxb = sb.tile([128, nch * 8], I16, tag="idxb", name="idxb",
                           padded_shape=[128, (KMAX * (WINB + 1) * NG) * 8])
            nc.sync.dma_start(idxb[:], t_idx[:, c0 * 8:(c0 + nch) * 8])
            drelb = sb.tile([128, nch], F16, tag="drelb", name="drelb",
                            padded_shape=[128, KMAX * (WINB + 1) * NG])
            nc.sync.dma_start(drelb[:], t_drel[:, c0:c0 + nch])
            aggp = {wl: ps_agg.tile([128, 128], F32, tag=f"aggp{wl}",
                                    name=f"aggp{wl}")
                    for wl in range(len(batch["wins"]))}
            for call in batch["calls"]:
                k, g, gid0 = call["k"], call["g"], call["gid0"]
                lc = gid0 - c0
                msg = msgp.tile([128, k * 128], F16, tag="msg", name="msg",
                                padded_shape=[128, KMAX * 128])
                nc.gpsimd.dma_gather(
                    out_ap=msg[:].rearrange("p (c e) -> p c e", e=HID),
                    in_ap=tables[layer][GBASES[g]:GBASES[g] + GSIZES[g], :],
                    idxs_ap=idxb[:, lc * 8:(lc + k) * 8],
                    num_idxs=k * 128, num_idxs_reg=k * 128, elem_size=HID,
                    single_packet=False, queue_num=qi % NQ)
                qi += 1
                if "iseq" not in parts:
                    nc.vector.tensor_tensor(out=dbg_acc[:], in0=dbg_acc[:],
                                            in1=msg[:, :128], op=AO.add)
                    continue
                scall = scp.tile([128, k * 128], F16, tag="scall", name="scall",
                                padded_shape=[128, KMAX * 128])
                drs = drelb[:, lc:lc + k]
                in0 = AP(iota_f[:].tensor, iota_f[:].offset,
                         [iota_f[:].ap[0], [0, k], iota_f[:].ap[1]])
                in1 = AP(drs.tensor, drs.offset,
                         [drs.ap[0], drs.ap[1], [0, 128]])
                nc.vector.tensor_tensor(
                    out=scall[:].rearrange("p (c e) -> p c e", e=128),
                    in0=in0, in1=in1, op=AO.is_equal)
                if "mm" not in parts:
                    nc.vector.tensor_tensor(out=dbg_acc[:], in0=dbg_acc[:],
                                            in1=msg[:, :128], op=AO.add)
                    nc.vector.tensor_tensor(out=dbg_acc[:], in0=dbg_acc[:],
                                            in1=scall[:, :128], op=AO.add)
                    continue
                for j, (w, st, sp) in enumerate(call["chunks"]):
                    wl = w % WINB
                    nc.tensor.matmul(
                        out=aggp[wl][:],
                        lhsT=scall[:, j * 128:(j + 1) * 128],
                        rhs=msg[:, j * 128:(j + 1) * 128],
                        start=st, stop=sp)
            if "mm" not in parts:
                continue
            for w in batch["wins"]:
                wl = w % WINB
                nc.scalar.activation(agg_big[:, ts(w)], aggp[wl][:],
                                     AF.Copy, scale=dinv_sb[:, w:w + 1])
                if "stats" not in parts:
                    continue
                sq = sb.tile([128, 128], F32, tag="sq", name="sqt")
                nc.vector.tensor_tensor(out=sq[:], in0=agg_big[:, ts(w)],
                                        in1=agg_big[:, ts(w)], op=AO.mult)
                nc.tensor.matmul(out=sum_ps[:], lhsT=ones_col[:],
                                 rhs=agg_big[:, ts(w)],
                                 start=(w == 0), stop=(w == TPC - 1))
                nc.tensor.matmul(out=sq_ps[:], lhsT=ones_col[:], rhs=sq[:],
                                 start=(w == 0), stop=(w == TPC - 1))

        if stage <= 3 and layer == 1:
            dbg_out(dbg_acc if dbg_acc is not None else agg_big)
            return
        # ---- BN stats all-reduce
        st_sb = sb.tile([1, 256], F32, tag="stv", name="stv")
        nc.vector.tensor_copy(st_sb[:, :128], sum_ps[:])
        nc.vector.tensor_copy(st_sb[:, 128:], sq_ps[:])
        nc.sync.dma_start(ar_in[:], st_sb[:])
        nc.gpsimd.collective_compute(
            "AllReduce", mybir.AluOpType.add, replica_groups=RG,
            ins=[ar_in.opt()], outs=[ar_outs[layer].opt()])
        gl_sb = sb.tile([1, 256], F32, tag="stv", name="glv")
        nc.sync.dma_start(gl_sb[:], ar_outs[layer][:])

        # ---- BN affine coefficients A, B [1, 128]
        stat = sb.tile([1, 128 * 6], F32, tag="bn", name="bn")
        mu, ex2, var, rs, A, B = (stat[:, i * 128:(i + 1) * 128]
                                  for i in range(6))
        nc.vector.tensor_scalar_mul(mu, gl_sb[:, :128], 1.0 / N_NODES)
        nc.vector.tensor_scalar_mul(ex2, gl_sb[:, 128:], 1.0 / N_NODES)
        nc.vector.tensor_tensor(out=var, in0=mu, in1=mu, op=AO.mult)
        nc.vector.tensor_tensor(out=var, in0=ex2, in1=var, op=AO.subtract)
        sd = sb.tile([1, 128], F32, tag="sd", name="sd")
        nc.scalar.activation(sd[:], var, AF.Sqrt, bias=eps_sb[:])
        nc.vector.reciprocal(rs, sd[:])
        nc.vector.tensor_tensor(out=A, in0=rs, in1=g_sb[layer][:], op=AO.mult)
        nc.vector.tensor_tensor(out=B, in0=mu, in1=A, op=AO.mult)
        nc.vector.tensor_tensor(out=B, in0=bt_sb[layer][:], in1=B,
                                op=AO.subtract)
        bca_ps = ps_b.tile([128, 128], F32, tag="mmp", name="bca")
        nc.tensor.matmul(out=bca_ps[:], lhsT=ones_row[:], rhs=A,
                         start=True, stop=True)
        A_bc = sb.tile([128, 128], F32, tag="abc", name="abc")
        nc.vector.tensor_copy(A_bc[:], bca_ps[:])
        bcb_ps = ps_b.tile([128, 128], F32, tag="mmp", name="bcb")
        nc.tensor.matmul(out=bcb_ps[:], lhsT=ones_row[:], rhs=B,
                         start=True, stop=True)
        B_bc = sb.tile([128, 128], F32, tag="bbc", name="bbc")
        nc.vector.tensor_copy(B_bc[:], bcb_ps[:])

        # ---- h = relu(A*agg + B) + h
        for t in range(TPC):
            y = sb.tile([128, 128], F32, tag="y", name="y")
            nc.vector.tensor_tensor(out=y[:], in0=agg_big[:, ts(t)],
                                    in1=A_bc[:], op=AO.mult)
            nc.vector.tensor_tensor(out=y[:], in0=y[:], in1=B_bc[:], op=AO.add)
            nc.vector.tensor_scalar_max(y[:], y[:], 0.0)
            nc.vector.tensor_tensor(out=h_big[:, ts(t)], in0=y[:],
                                    in1=h_big[:, ts(t)], op=AO.add)
        if stage <= 4 and layer == 1:
            dbg_out(h_big)
            return
        if layer < 3:
            emit_table(W_sb[layer + 1], tables[layer + 1])

    # ------------------------------------------------------------------
    # head: out = tanh(relu(BN(h3 @ Wf1)) @ Wf2 + bf2)
    # ------------------------------------------------------------------
    fsum_ps = ps_st.tile([1, 32], F32, tag="sum", name="fsum")
    fsq_ps = ps_st.tile([1, 32], F32, tag="sq", name="fsq")
    for t in range(TPC):
        trp = ps_a.tile([128, 128], F32, tag="trp", name="trp_h")
        nc.tensor.transpose(out=trp[:], in_=h_big[:, ts(t)], identity=ident[:])
        ht = sb.tile([128, 128], F32, tag="ht", name="ht_h")
        nc.vector.tensor_copy(ht[:], trp[:])
        fp = ps_b.tile([128, 32], F32, tag="mmp", name="fp")
        nc.tensor.matmul(out=fp[:], lhsT=ht[:], rhs=Wf1_sb[:],
                         start=True, stop=True)
        fs = slice(t * 32, (t + 1) * 32)
        nc.vector.tensor_scalar(out=fpre_big[:, fs], in0=fp[:],
                                scalar1=msk_sb[:, t:t + 1], scalar2=None,
                                op0=AO.mult)
        sq = sb.tile([128, 32], F32, tag="sq32", name="sq32")
        nc.vector.tensor_tensor(out=sq[:], in0=fpre_big[:, fs],
                                in1=fpre_big[:, fs], op=AO.mult)
        nc.tensor.matmul(out=fsum_ps[:], lhsT=ones_col[:], rhs=fpre_big[:, fs],
                         start=(t == 0), stop=(t == TPC - 1))
        nc.tensor.matmul(out=fsq_ps[:], lhsT=ones_col[:], rhs=sq[:],
                         start=(t == 0), stop=(t == TPC - 1))

    fst = sb.tile([1, 64], F32, tag="fst", name="fst")
    nc.vector.tensor_copy(fst[:, :32], fsum_ps[:])
    nc.vector.tensor_copy(fst[:, 32:], fsq_ps[:])
    nc.sync.dma_start(ar_in_f[:], fst[:])
    nc.gpsimd.collective_compute(
        "AllReduce", mybir.AluOpType.add, replica_groups=RG,
        ins=[ar_in_f.opt()], outs=[ar_out_f.opt()])
    fgl = sb.tile([1, 64], F32, tag="fst", name="fgl")
    nc.sync.dma_start(fgl[:], ar_out_f[:])

    fstat = sb.tile([1, 32 * 6], F32, tag="bn", name="fbn")
    mu, ex2, var, rs, A, B = (fstat[:, i * 32:(i + 1) * 32] for i in range(6))
    AO, AF = mybir.AluOpType, mybir.ActivationFunctionType
    nc.vector.tensor_scalar_mul(mu, fgl[:, :32], 1.0 / N_NODES)
    nc.vector.tensor_scalar_mul(ex2, fgl[:, 32:], 1.0 / N_NODES)
    nc.vector.tensor_tensor(out=var, in0=mu, in1=mu, op=AO.mult)
    nc.vector.tensor_tensor(out=var, in0=ex2, in1=var, op=AO.subtract)
    fsd = sb.tile([1, 32], F32, tag="sd", name="fsd")
    nc.scalar.activation(fsd[:], var, AF.Sqrt, bias=eps_sb[:])
    nc.vector.reciprocal(rs, fsd[:])
    nc.vector.tensor_tensor(out=A, in0=rs, in1=gf_sb[:], op=AO.mult)
    nc.vector.tensor_tensor(out=B, in0=mu, in1=A, op=AO.mult)
    nc.vector.tensor_tensor(out=B, in0=btf_sb[:], in1=B, op=AO.subtract)
    fa_ps = ps_b.tile([128, 32], F32, tag="mmp", name="fa")
    nc.tensor.matmul(out=fa_ps[:], lhsT=ones_row[:], rhs=A, start=True, stop=True)
    Af_bc = sb.tile([128, 32], F32, tag="abc", name="fabc")
    nc.vector.tensor_copy(Af_bc[:], fa_ps[:])
    fb_ps = ps_b.tile([128, 32], F32, tag="mmp", name="fb")
    nc.tensor.matmul(out=fb_ps[:], lhsT=ones_row[:], rhs=B, start=True, stop=True)
    Bf_bc = sb.tile([128, 32], F32, tag="bbc", name="fbbc")
    nc.vector.tensor_copy(Bf_bc[:], fb_ps[:])

    for t in range(TPC):
        fs = slice(t * 32, (t + 1) * 32)
        f = sb.tile([128, 32], F32, tag="f", name="f")
        nc.vector.tensor_tensor(out=f[:], in0=fpre_big[:, fs], in1=Af_bc[:],
                                op=AO.mult)
        nc.vector.tensor_tensor(out=f[:], in0=f[:], in1=Bf_bc[:], op=AO.add)
        nc.vector.tensor_scalar_max(f[:], f[:], 0.0)
        ftr_ps = ps_a.tile([32, 128], F32, tag="trp", name="ftr")
        nc.tensor.transpose(out=ftr_ps[:], in_=f[:], identity=ident[:])
        ftr = sb.tile([32, 128], F32, tag="ht", name="ftrs")
        nc.vector.tensor_copy(ftr[:], ftr_ps[:])
        op = ps_b.tile([2, 128], F32, tag="mmp", name="op")
        nc.tensor.matmul(out=op[:], lhsT=Wf2_sb[:], rhs=ftr[:],
                         start=True, stop=True)
        ot = sb.tile([2, 128], F32, tag="ot", name="ot")
        nc.scalar.activation(ot[:], op[:], AF.Tanh, bias=bf2_sb[:])
        nc.sync.dma_start(t_out[:, ts(t)], ot[:])


# ----------------------------------------------------------------------------
# Public entry point
# ----------------------------------------------------------------------------

_CACHE = {}


def _get_compiled(edge_index, x):
    key = hash((edge_index.tobytes(), x.shape))
    if key not in _CACHE:
        plan, streams = _preprocess(edge_index, x)
        nc = _build_program(plan)
        _CACHE.clear()
        _CACHE[key] = (nc, streams)
    return _CACHE[key]


def _in_maps(streams, kw):
    rep = dict(
        We=np.asarray(kw["We"], np.float32),
        W1=np.asarray(kw["W1"], np.float32),
        W2=np.asarray(kw["W2"], np.float32),
        W3=np.asarray(kw["W3"], np.float32),
        Wf1=np.asarray(kw["Wf1"], np.float32),
        Wf2=np.asarray(kw["Wf2"], np.float32),
        be_bc=np.tile(np.asarray(kw["be"], np.float32)[None, :], (128, 1)),
        g1=np.asarray(kw["g1"], np.float32)[None, :],
        bt1=np.asarray(kw["bt1"], np.float32)[None, :],
        g2=np.asarray(kw["g2"], np.float32)[None, :],
        bt2=np.asarray(kw["bt2"], np.float32)[None, :],
        g3=np.asarray(kw["g3"], np.float32)[None, :],
        bt3=np.asarray(kw["bt3"], np.float32)[None, :],
        gf=np.asarray(kw["gf"], np.float32)[None, :],
        btf=np.asarray(kw["btf"], np.float32)[None, :],
        bf2c=np.asarray(kw["bf2"], np.float32)[:, None],
    )
    return [dict(rep, **streams[c]) for c in range(NCORES)]


def run(trace=False, **kw):
    x = np.asarray(kw["x"], np.float32)
    edge_index = np.asarray(kw["edge_index"], np.int32)
    nc, streams = _get_compiled(edge_index, x)
    res = run_bass_kernel_spmd(nc, _in_maps(streams, kw),
                               core_ids=list(range(NCORES)), trace=trace)
    shards = [res.results[c]["out"][:, :NSH].T for c in range(NCORES)]
    out = np.ascontiguousarray(np.concatenate(shards, 0))
    return out, res


def kernel(**kw):
    out, _ = run(trace=False, **kw)
    return out



# revision 22
# speedup vs baseline: 1.4720x; 1.4720x over previous
"""Trainium2 Bass kernel for nn_EnhancedGNN (3-block GCN + BN + MLP head).

Strategy (8 NeuronCores, node-partitioned graph parallel):
  - Nodes are partitioned contiguously across the 8 cores (12500 each,
    padded to 12544 = 98*128). Each core owns all edges whose dst lies in
    its partition (plus the self-loops of its own nodes).
  - Per GCN layer, each core holds the full feature table
    H~ = dinv * (h @ W) (replicated via AllGather, [100352, 128] f32 in
    DRAM). Messages are fetched with dma_gather (random 512B rows).
  - Segment-sum by dst is a one-hot matmul: for each chunk of 128
    dst-sorted messages, S[msg, dstslot] = (dstrel[msg] == iota[slot])
    built on DVE, then PSUM += S^T @ msgs on TensorE. dinv[dst] is applied
    on PSUM evacuation (ScalarE per-partition scale); dinv[src] is folded
    into the table rows; GCN bias is absorbed by the following BatchNorm.
  - BN statistics are computed with ones-matmuls and a tiny AllReduce.
"""

import numpy as np

import concourse.bacc as bacc
import concourse.mybir as mybir
import concourse.tile as tile
from concourse.bass import AP
from concourse.bass_utils import run_bass_kernel_spmd
from concourse.masks import make_identity

F32 = mybir.dt.float32
F16 = mybir.dt.float16
I16 = mybir.dt.int16
I32 = mybir.dt.int32

N_NODES = 100000
HID = 128
NCORES = 8
EPS = 1e-5
RG = [list(range(NCORES))]
WINB = 4                         # windows per PSUM batch ([128, 512] tile)
KMAX = 16                        # chunks per dma_gather call
NQ = 4                           # SWDGE queues for parallel gather desc-gen


def _set_config(n_nodes=100000, group=32768):
    """Derive all sharding constants (module globals) from the node count."""
    global N_NODES, NSH, TPC, NPAD, TROWS, GROUP, GBASES, GSIZES, NG
    N_NODES = n_nodes
    NSH = N_NODES // NCORES
    TPC = (NSH + 127) // 128
    NPAD = TPC * 128
    TROWS = NCORES * NPAD
    GROUP = group
    GBASES = list(range(0, TROWS, GROUP))
    GSIZES = [min(GROUP, TROWS - b) for b in GBASES]
    NG = len(GBASES)


_set_config()


# ----------------------------------------------------------------------------
# Host-side preprocessing: edge bucketing + per-core streams
# ----------------------------------------------------------------------------

def _preprocess(edge_index, x):
    src = edge_index[0].astype(np.int64)
    dst = edge_index[1].astype(np.int64)
    loop = np.arange(N_NODES, dtype=np.int64)
    src = np.concatenate([src, loop])
    dst = np.concatenate([dst, loop])

    deg = np.bincount(dst, minlength=N_NODES).astype(np.float32)
    dinv = (1.0 / np.sqrt(deg)).astype(np.float32)

    srow = (src // NSH) * NPAD + (src % NSH)     # table row id
    core = dst // NSH
    dloc = dst % NSH
    win = dloc // 128
    drel = dloc % 128
    grp = srow // GROUP

    counts = np.zeros((NCORES, TPC, NG), np.int64)
    percore = []
    for c in range(NCORES):
        m = core == c
        s_c, w_c, g_c, d_c = srow[m], win[m], grp[m], drel[m]
        o = np.lexsort((d_c, g_c, w_c))
        s_c, w_c, g_c, d_c = s_c[o], w_c[o], g_c[o], d_c[o]
        cnt = np.zeros((TPC, NG), np.int64)
        np.add.at(cnt, (w_c, g_c), 1)
        counts[c] = cnt
        percore.append((s_c, d_c, cnt))
    assert (counts.max(0).sum(1) > 0).all()

    # ---- static call plan shared by every core (buckets are (batch, g);
    # chunks may span windows; drel carries 128*(w - w0) so the S build per
    # (chunk, window) uses a shifted iota)
    batches = []
    bucket_info = []                  # (b, g) -> dict(gid0, nch, pairs)
    gid = 0
    nb = -(-TPC // WINB)
    for b in range(nb):
        wins = list(range(b * WINB, min((b + 1) * WINB, TPC)))
        w0 = wins[0]
        # per-window matmul pair counts to set start/stop flags
        pair_seq = {w: [] for w in wins}           # w -> [(bucket_idx, chunk)]
        bstart = len(bucket_info)
        chunk0 = gid
        for g in range(NG):
            # per-core segment boundaries of each window inside this bucket
            seg = np.zeros((NCORES, len(wins) + 1), np.int64)
            for c in range(NCORES):
                seg[c, 1:] = np.cumsum([counts[c][w, g] for w in wins])
            cmaxB = int(seg[:, -1].max())
            nch = -(-cmaxB // 128)
            if nch == 0:
                continue
            pairs = [[] for _ in range(nch)]
            for j in range(nch):
                lo, hi = 128 * j, 128 * (j + 1)
                for wi, w in enumerate(wins):
                    # window wi's rows span [seg[c,wi], seg[c,wi+1]) per core
                    if any(seg[c, wi] < hi and seg[c, wi + 1] > lo
                           for c in range(NCORES)):
                        pairs[j].append(w)
                for w in pairs[j]:
                    pair_seq[w].append((len(bucket_info), j))
            bucket_info.append(dict(b=b, g=g, gid0=gid, nch=nch, seg=seg))
            gid += nch
        # start/stop assignment
        flags = {}                                  # (bi, j, w) -> [st, sp]
        for w in wins:
            sq = pair_seq[w]
            assert sq, f"window {w} has no messages"
            for i, (bi, j) in enumerate(sq):
                flags[(bi, j, w)] = (i == 0, i == len(sq) - 1)
        calls = []
        for bi in range(bstart, len(bucket_info)):
            info = bucket_info[bi]
            g, nch, gid0 = info["g"], info["nch"], info["gid0"]
            for i0 in range(0, nch, KMAX):
                k = min(KMAX, nch - i0)
                ch = []
                for j in range(i0, i0 + k):
                    # per-chunk list of (window, start, stop)
                    prs = []
                    for w in ([] if bi >= len(bucket_info) else
                              [w for w in wins
                               if (bi, j, w) in flags]):
                        st, sp = flags[(bi, j, w)]
                        prs.append((w, st, sp))
                    ch.append(prs)
                calls.append(dict(g=g, k=k, gid0=gid0 + i0, chunks=ch))
        batches.append(dict(wins=wins, calls=calls, chunk0=chunk0,
                            nchunks=gid - chunk0))
    tot_chunks = gid

    # ---- per-core streams
    streams = []
    for c in range(NCORES):
        s_c, d_c, cnt = percore[c]
        flat = cnt.reshape(-1)
        offs = np.concatenate([[0], np.cumsum(flat)[:-1]]).reshape(TPC, NG)
        idx_stream = np.zeros((tot_chunks, 128), np.int16)
        drel_stream = np.full((tot_chunks, 128), -1.0, np.float32)
        for info in bucket_info:
            b, g, gid0, nch = info["b"], info["g"], info["gid0"], info["nch"]
            wins = batches[b]["wins"]
            w0 = wins[0]
            ncap = nch * 128
            vals = np.zeros(ncap, np.int64)
            dr = np.full(ncap, -1.0, np.float32)
            pos = 0
            for w in wins:
                n = cnt[w, g]
                o0 = offs[w, g]
                vals[pos:pos + n] = s_c[o0:o0 + n] - GBASES[g]
                dr[pos:pos + n] = d_c[o0:o0 + n] + 128 * (w - w0)
                pos += n
            gids = range(gid0, gid0 + nch)
            idx_stream[gids] = vals.reshape(-1, 128).astype(np.int16)
            drel_stream[gids] = dr.reshape(-1, 128)
        # wrap indices: chunk j, msg p -> [p % 16, j*8 + p//16], replicate x8
        idx16 = idx_stream.reshape(tot_chunks * 8, 16).T          # [16, 8*T]
        idx_wrapped = np.tile(idx16, (8, 1)).copy()               # [128, 8*T]
        drel_cols = np.ascontiguousarray(drel_stream.T).astype(np.float16)
        dv = np.zeros(NPAD, np.float32)
        dv[:NSH] = dinv[c * NSH:(c + 1) * NSH]
        dinv_cols = np.ascontiguousarray(dv.reshape(TPC, 128).T)  # [128, TPC]
        mk = np.zeros(NPAD, np.float32)
        mk[:NSH] = 1.0
        msk_cols = np.ascontiguousarray(mk.reshape(TPC, 128).T)
        xp = np.zeros((NPAD, 2), np.float32)
        xp[:NSH] = x[c * NSH:(c + 1) * NSH]
        xT = np.ascontiguousarray(xp.T)                           # [2, NPAD]
        streams.append(dict(idxs=idx_wrapped, drel=drel_cols,
                            dinv=dinv_cols, msk=msk_cols, xT=xT))

    plan = dict(batches=batches, tot_chunks=tot_chunks,
                tot_cols=tot_chunks * 8)
    return plan, streams


# ----------------------------------------------------------------------------
# Device program
# ----------------------------------------------------------------------------

def _build_program(plan):
    nc = bacc.Bacc("TRN2", target_bir_lowering=False, debug=False,
                   enable_asserts=True, num_devices=NCORES,
                   num_swdge_queues=NQ)

    def din(name, shape, dt=F32):
        return nc.dram_tensor(name, list(shape), dt, kind="ExternalInput").ap()

    t_idx = din("idxs", [128, plan["tot_cols"]], I16)
    t_drel = din("drel", [128, plan["tot_chunks"]], F16)
    t_dinv = din("dinv", [128, TPC])
    t_msk = din("msk", [128, TPC])
    t_xT = din("xT", [2, NPAD])
    t_We = din("We", [2, HID])
    t_W = {1: din("W1", [HID, HID]), 2: din("W2", [HID, HID]),
           3: din("W3", [HID, HID])}
    t_Wf1 = din("Wf1", [HID, 32])
    t_Wf2 = din("Wf2", [32, 2])
    t_bebc = din("be_bc", [128, HID])
    t_g = {1: din("g1", [1, HID]), 2: din("g2", [1, HID]), 3: din("g3", [1, HID])}
    t_bt = {1: din("bt1", [1, HID]), 2: din("bt2", [1, HID]),
            3: din("bt3", [1, HID])}
    t_gf = din("gf", [1, 32])
    t_btf = din("btf", [1, 32])
    t_bf2 = din("bf2c", [2, 1])
    t_out = nc.dram_tensor("out", [2, NPAD], F32, kind="ExternalOutput").ap()

    from contextlib import ExitStack
    with tile.TileContext(nc) as tc, ExitStack() as st:
        cst = st.enter_context(tc.tile_pool(name="cst", bufs=1))
        sb = st.enter_context(tc.tile_pool(name="sb", bufs=2))
        scp = st.enter_context(tc.tile_pool(name="scp", bufs=4))
        msgp = st.enter_context(tc.tile_pool(name="msgp", bufs=8))
        ps_agg = st.enter_context(tc.tile_pool(name="ps_agg", bufs=1, space="PSUM"))
        ps_st = st.enter_context(tc.tile_pool(name="ps_st", bufs=1, space="PSUM"))
        ps_a = st.enter_context(tc.tile_pool(name="ps_a", bufs=1, space="PSUM"))
        ps_b = st.enter_context(tc.tile_pool(name="ps_b", bufs=1, space="PSUM"))
        dr = st.enter_context(tc.tile_pool(name="dr", bufs=1, space="DRAM"))
        _emit(nc, tc, plan, locals())
    nc.compile()
    return nc


def _emit(nc, tc, plan, pools):
    cst, sb, msgp = pools["cst"], pools["sb"], pools["msgp"]
    scp = pools["scp"]
    ps_agg, ps_st = pools["ps_agg"], pools["ps_st"]
    ps_a, ps_b, dr = pools["ps_a"], pools["ps_b"], pools["dr"]
    t_idx, t_drel = pools["t_idx"], pools["t_drel"]
    t_dinv, t_xT, t_We = pools["t_dinv"], pools["t_xT"], pools["t_We"]
    t_W, t_Wf1, t_Wf2 = pools["t_W"], pools["t_Wf1"], pools["t_Wf2"]
    t_bebc, t_g, t_bt = pools["t_bebc"], pools["t_g"], pools["t_bt"]
    t_gf, t_btf, t_bf2 = pools["t_gf"], pools["t_btf"], pools["t_bf2"]
    t_out = pools["t_out"]
    AO, AF = mybir.AluOpType, mybir.ActivationFunctionType

    # ---- constants
    iota_i = cst.tile([128, WINB * 128], I32)
    nc.gpsimd.iota(iota_i[:], pattern=[[1, WINB * 128]], base=0,
                   channel_multiplier=0)
    iota_f = cst.tile([128, WINB * 128], F16)
    nc.vector.tensor_copy(iota_f[:], iota_i[:])

    # whole-program gather streams resident in SBUF (layer-independent);
    # avoids per-batch HWDGE loads that stall the Pool gather pipeline
    idx_all = cst.tile([128, plan["tot_cols"]], I16)
    nc.sync.dma_start(idx_all[:], pools["t_idx"][:])
    drel_all = cst.tile([128, plan["tot_chunks"]], F16)
    nc.sync.dma_start(drel_all[:], pools["t_drel"][:])
    ident = cst.tile([128, 128], F32)
    make_identity(nc, ident[:])
    ones_col = cst.tile([128, 1], F32)
    nc.vector.memset(ones_col[:], 1.0)
    ones_row = cst.tile([1, 128], F32)
    nc.vector.memset(ones_row[:], 1.0)
    eps_sb = cst.tile([1, 1], F32)
    nc.vector.memset(eps_sb[:], EPS)

    def load_const(t, shape, dt=F32):
        tl = cst.tile(shape, dt, name=f"c_{t.tensor.name}")
        nc.sync.dma_start(tl[:], t[:])
        return tl

    dinv_sb = load_const(t_dinv, [128, TPC])
    msk_sb = load_const(pools["t_msk"], [128, TPC])
    We_sb = load_const(t_We, [2, HID])
    W_sb = {i: load_const(t_W[i], [HID, HID]) for i in (1, 2, 3)}
    Wf1_sb = load_const(t_Wf1, [HID, 32])
    Wf2_sb = load_const(t_Wf2, [32, 2])
    bebc_sb = load_const(t_bebc, [128, HID])
    g_sb = {i: load_const(t_g[i], [1, HID]) for i in (1, 2, 3)}
    bt_sb = {i: load_const(t_bt[i], [1, HID]) for i in (1, 2, 3)}
    gf_sb = load_const(t_gf, [1, 32])
    btf_sb = load_const(t_btf, [1, 32])
    bf2_sb = load_const(t_bf2, [2, 1], F32)

    h_big = cst.tile([128, NPAD], F32)       # node-major h (residual carrier)
    agg_big = cst.tile([128, NPAD], F32)     # node-major aggregation output
    fpre_big = cst.tile([128, TPC * 32], F32)

    tshard = dr.tile([NPAD, HID], F16, name="tshard")
    tables = {i: dr.tile([TROWS, HID], F16, addr_space="Shared",
                         name=f"table{i}") for i in (1, 2, 3)}
    ar_in = dr.tile([1, 256], F32, name="ar_in")
    ar_outs = {i: dr.tile([1, 256], F32, addr_space="Shared",
                          name=f"ar_out{i}") for i in (1, 2, 3)}
    ar_in_f = dr.tile([1, 64], F32, name="ar_in_f")
    ar_out_f = dr.tile([1, 64], F32, addr_space="Shared", name="ar_out_f")

    def ts(t):
        return slice(t * 128, (t + 1) * 128)

    stage = plan.get("stage", 99)

    def dbg_out(src):
        w = min(NPAD, src.shape[1])
        nc.sync.dma_start(t_out[:, :w], src[:2, :w])

    def emit_table(Wnext_sb, table):
        """tshard <- dinv * (h @ Wnext), then AllGather into table."""
        for t in range(TPC):
            trp = ps_a.tile([128, 128], F32, tag="trp", name="trp")
            nc.tensor.transpose(out=trp[:], in_=h_big[:, ts(t)], identity=ident[:])
            ht = sb.tile([128, 128], F32, tag="ht", name="ht")
            nc.vector.tensor_copy(ht[:], trp[:])
            mmp = ps_b.tile([128, 128], F32, tag="mmp", name="mmp")
            nc.tensor.matmul(out=mmp[:], lhsT=ht[:], rhs=Wnext_sb[:],
                             start=True, stop=True)
            tsh = sb.tile([128, 128], F16, tag="tsh", name="tsh")
            nc.scalar.activation(tsh[:], mmp[:], AF.Copy,
                                 scale=dinv_sb[:, t:t + 1])
            nc.sync.dma_start(tshard[ts(t), :], tsh[:])
        nc.gpsimd.collective_compute(
            "AllGather", mybir.AluOpType.bypass, replica_groups=RG,
            ins=[tshard.opt()], outs=[table.opt()])

    # ------------------------------------------------------------------
    # embed: h = relu(x @ We + be)
    # ------------------------------------------------------------------
    if plan.get("skip_embed"):
        nc.vector.memset(h_big[:], 0.25)
    else:
        for t in range(TPC):
            xt = sb.tile([2, 128], F32, tag="xt", name="xt")
            nc.sync.dma_start(xt[:], t_xT[:, ts(t)])
            mmp = ps_b.tile([128, 128], F32, tag="mmp", name="mmp_e")
            nc.tensor.matmul(out=mmp[:], lhsT=xt[:], rhs=We_sb[:],
                             start=True, stop=True)
            nc.vector.tensor_tensor(out=h_big[:, ts(t)], in0=mmp[:],
                                    in1=bebc_sb[:], op=AO.add)
            nc.vector.tensor_scalar_max(h_big[:, ts(t)], h_big[:, ts(t)], 0.0)
    if stage <= 1:
        dbg_out(h_big)
        return
    if plan.get("simple_table"):
        for t in range(TPC):
            tsh = sb.tile([128, 128], F16, tag="tsh", name="tsh_s")
            nc.vector.tensor_copy(tsh[:], h_big[:, ts(t)])
            nc.sync.dma_start(tshard[ts(t), :], tsh[:])
        nc.gpsimd.collective_compute(
            "AllGather", mybir.AluOpType.bypass, replica_groups=RG,
            ins=[tshard.opt()], outs=[tables[1].opt()])
    else:
        emit_table(W_sb[1], tables[1])
    if stage <= 2:
        tb = sb.tile([2, 128], F32, tag="dbg", name="dbgt")
        nc.sync.dma_start(tb[:], tables[1][:2, :])
        zz = sb.tile([2, NPAD], F32, tag="dbg2", name="dbgz")
        nc.vector.memset(zz[:], 0.0)
        nc.vector.tensor_copy(zz[:, :128], tb[:])
        dbg_out(zz)
        return

    # ------------------------------------------------------------------
    # 3 GCN blocks
    # ------------------------------------------------------------------
    qi = 0
    for layer in (1, 2, 3):
        sum_ps = ps_st.tile([1, 128], F32, tag="sum", name=f"sum{layer}")
        sq_ps = ps_st.tile([1, 128], F32, tag="sq", name=f"sq{layer}")
        for batch in plan["batches"]:
            w0 = batch["wins"][0]
            aggp = {wl: ps_agg.tile([128, 128], F32, tag=f"aggp{wl}",
                                    name=f"aggp{wl}")
                    for wl in range(len(batch["wins"]))}
            for call in batch["calls"]:
                k, g, gid0 = call["k"], call["g"], call["gid0"]
                msg = msgp.tile([128, k * 128], F16, tag="msg", name="msg",
                                padded_shape=[128, KMAX * 128])
                nc.gpsimd.dma_gather(
                    out_ap=msg[:].rearrange("p (c e) -> p c e", e=HID),
                    in_ap=tables[layer][GBASES[g]:GBASES[g] + GSIZES[g], :],
                    idxs_ap=idx_all[:, gid0 * 8:(gid0 + k) * 8],
                    num_idxs=k * 128, num_idxs_reg=k * 128, elem_size=HID,
                    single_packet=False, queue_num=qi % NQ)
                qi += 1
                for j, prs in enumerate(call["chunks"]):
                    ch = gid0 + j
                    for (w, st, sp) in prs:
                        wl = w - w0
                        scall = scp.tile([128, 128], F16, tag="scall",
                                         name="scall")
                        drs = drel_all[:, ch:ch + 1]
                        in1 = AP(drs.tensor, drs.offset,
                                 [drs.ap[0], [0, 128]])
                        nc.vector.tensor_tensor(
                            out=scall[:], in0=iota_f[:, wl * 128:(wl + 1) * 128],
                            in1=in1, op=AO.is_equal)
                        nc.tensor.matmul(
                            out=aggp[wl][:], lhsT=scall[:],
                            rhs=msg[:, j * 128:(j + 1) * 128],
                            start=st, stop=sp)
            for w in batch["wins"]:
                wl = w - w0
                nc.scalar.activation(agg_big[:, ts(w)], aggp[wl][:],
                                     AF.Copy, scale=dinv_sb[:, w:w + 1])
                sq = sb.tile([128, 128], F32, tag="sq", name="sqt")
                nc.vector.tensor_tensor(out=sq[:], in0=agg_big[:, ts(w)],
                                        in1=agg_big[:, ts(w)], op=AO.mult)
                nc.tensor.matmul(out=sum_ps[:], lhsT=ones_col[:],
                                 rhs=agg_big[:, ts(w)],
                                 start=(w == 0), stop=(w == TPC - 1))
                nc.tensor.matmul(out=sq_ps[:], lhsT=ones_col[:], rhs=sq[:],
                                 start=(w == 0), stop=(w == TPC - 1))

        # ---- BN stats all-reduce
        st_sb = sb.tile([1, 256], F32, tag="stv", name="stv")
        nc.vector.tensor_copy(st_sb[:, :128], sum_ps[:])
        nc.vector.tensor_copy(st_sb[:, 128:], sq_ps[:])
        nc.sync.dma_start(ar_in[:], st_sb[:])
        nc.gpsimd.collective_compute(
            "AllReduce", mybir.AluOpType.add, replica_groups=RG,
            ins=[ar_in.opt()], outs=[ar_outs[layer].opt()])
        gl_sb = sb.tile([1, 256], F32, tag="stv", name="glv")
        nc.sync.dma_start(gl_sb[:], ar_outs[layer][:])

        # ---- BN affine coefficients A, B [1, 128]
        stat = sb.tile([1, 128 * 6], F32, tag="bn", name="bn")
        mu, ex2, var, rs, A, B = (stat[:, i * 128:(i + 1) * 128]
                                  for i in range(6))
        nc.vector.tensor_scalar_mul(mu, gl_sb[:, :128], 1.0 / N_NODES)
        nc.vector.tensor_scalar_mul(ex2, gl_sb[:, 128:], 1.0 / N_NODES)
        nc.vector.tensor_tensor(out=var, in0=mu, in1=mu, op=AO.mult)
        nc.vector.tensor_tensor(out=var, in0=ex2, in1=var, op=AO.subtract)
        sd = sb.tile([1, 128], F32, tag="sd", name="sd")
        nc.scalar.activation(sd[:], var, AF.Sqrt, bias=eps_sb[:])
        nc.vector.reciprocal(rs, sd[:])
        nc.vector.tensor_tensor(out=A, in0=rs, in1=g_sb[layer][:], op=AO.mult)
        nc.vector.tensor_tensor(out=B, in0=mu, in1=A, op=AO.mult)
        nc.vector.tensor_tensor(out=B, in0=bt_sb[layer][:], in1=B,
                                op=AO.subtract)
        bca_ps = ps_b.tile([128, 128], F32, tag="mmp", name="bca")
        nc.tensor.matmul(out=bca_ps[:], lhsT=ones_row[:], rhs=A,
                         start=True, stop=True)
        A_bc = sb.tile([128, 128], F32, tag="abc", name="abc")
        nc.vector.tensor_copy(A_bc[:], bca_ps[:])
        bcb_ps = ps_b.tile([128, 128], F32, tag="mmp", name="bcb")
        nc.tensor.matmul(out=bcb_ps[:], lhsT=ones_row[:], rhs=B,
                         start=True, stop=True)
        B_bc = sb.tile([128, 128], F32, tag="bbc", name="bbc")
        nc.vector.tensor_copy(B_bc[:], bcb_ps[:])

        # ---- h = relu(A*agg + B) + h
        for t in range(TPC):
            y = sb.tile([128, 128], F32, tag="y", name="y")
            nc.vector.tensor_tensor(out=y[:], in0=agg_big[:, ts(t)],
                                    in1=A_bc[:], op=AO.mult)
            nc.vector.tensor_tensor(out=y[:], in0=y[:], in1=B_bc[:], op=AO.add)
            nc.vector.tensor_scalar_max(y[:], y[:], 0.0)
            nc.vector.tensor_tensor(out=h_big[:, ts(t)], in0=y[:],
                                    in1=h_big[:, ts(t)], op=AO.add)
        if stage <= 4 and layer == 1:
            dbg_out(h_big)
            return
        if layer < 3:
            emit_table(W_sb[layer + 1], tables[layer + 1])

    # ------------------------------------------------------------------
    # head: out = tanh(relu(BN(h3 @ Wf1)) @ Wf2 + bf2)
    # ------------------------------------------------------------------
    fsum_ps = ps_st.tile([1, 32], F32, tag="sum", name="fsum")
    fsq_ps = ps_st.tile([1, 32], F32, tag="sq", name="fsq")
    for t in range(TPC):
        trp = ps_a.tile([128, 128], F32, tag="trp", name="trp_h")
        nc.tensor.transpose(out=trp[:], in_=h_big[:, ts(t)], identity=ident[:])
        ht = sb.tile([128, 128], F32, tag="ht", name="ht_h")
        nc.vector.tensor_copy(ht[:], trp[:])
        fp = ps_b.tile([128, 32], F32, tag="mmp", name="fp")
        nc.tensor.matmul(out=fp[:], lhsT=ht[:], rhs=Wf1_sb[:],
                         start=True, stop=True)
        fs = slice(t * 32, (t + 1) * 32)
        nc.vector.tensor_scalar(out=fpre_big[:, fs], in0=fp[:],
                                scalar1=msk_sb[:, t:t + 1], scalar2=None,
                                op0=AO.mult)
        sq = sb.tile([128, 32], F32, tag="sq32", name="sq32")
        nc.vector.tensor_tensor(out=sq[:], in0=fpre_big[:, fs],
                                in1=fpre_big[:, fs], op=AO.mult)
        nc.tensor.matmul(out=fsum_ps[:], lhsT=ones_col[:], rhs=fpre_big[:, fs],
                         start=(t == 0), stop=(t == TPC - 1))
        nc.tensor.matmul(out=fsq_ps[:], lhsT=ones_col[:], rhs=sq[:],
                         start=(t == 0), stop=(t == TPC - 1))

    fst = sb.tile([1, 64], F32, tag="fst", name="fst")
    nc.vector.tensor_copy(fst[:, :32], fsum_ps[:])
    nc.vector.tensor_copy(fst[:, 32:], fsq_ps[:])
    nc.sync.dma_start(ar_in_f[:], fst[:])
    nc.gpsimd.collective_compute(
        "AllReduce", mybir.AluOpType.add, replica_groups=RG,
        ins=[ar_in_f.opt()], outs=[ar_out_f.opt()])
    fgl = sb.tile([1, 64], F32, tag="fst", name="fgl")
    nc.sync.dma_start(fgl[:], ar_out_f[:])

    fstat = sb.tile([1, 32 * 6], F32, tag="bn", name="fbn")
    mu, ex2, var, rs, A, B = (fstat[:, i * 32:(i + 1) * 32] for i in range(6))
    AO, AF = mybir.AluOpType, mybir.ActivationFunctionType
    nc.vector.tensor_scalar_mul(mu, fgl[:, :32], 1.0 / N_NODES)
    nc.vector.tensor_scalar_mul(ex2, fgl[:, 32:], 1.0 / N_NODES)
    nc.vector.tensor_tensor(out=var, in0=mu, in1=mu, op=AO.mult)
    nc.vector.tensor_tensor(out=var, in0=ex2, in1=var, op=AO.subtract)
    fsd = sb.tile([1, 32], F32, tag="sd", name="fsd")
    nc.scalar.activation(fsd[:], var, AF.Sqrt, bias=eps_sb[:])
    nc.vector.reciprocal(rs, fsd[:])
    nc.vector.tensor_tensor(out=A, in0=rs, in1=gf_sb[:], op=AO.mult)
    nc.vector.tensor_tensor(out=B, in0=mu, in1=A, op=AO.mult)
    nc.vector.tensor_tensor(out=B, in0=btf_sb[:], in1=B, op=AO.subtract)
    fa_ps = ps_b.tile([128, 32], F32, tag="mmp", name="fa")
    nc.tensor.matmul(out=fa_ps[:], lhsT=ones_row[:], rhs=A, start=True, stop=True)
    Af_bc = sb.tile([128, 32], F32, tag="abc", name="fabc")
    nc.vector.tensor_copy(Af_bc[:], fa_ps[:])
    fb_ps = ps_b.tile([128, 32], F32, tag="mmp", name="fb")
    nc.tensor.matmul(out=fb_ps[:], lhsT=ones_row[:], rhs=B, start=True, stop=True)
    Bf_bc = sb.tile([128, 32], F32, tag="bbc", name="fbbc")
    nc.vector.tensor_copy(Bf_bc[:], fb_ps[:])

    for t in range(TPC):
        fs = slice(t * 32, (t + 1) * 32)
        f = sb.tile([128, 32], F32, tag="f", name="f")
        nc.vector.tensor_tensor(out=f[:], in0=fpre_big[:, fs], in1=Af_bc[:],
                                op=AO.mult)
        nc.vector.tensor_tensor(out=f[:], in0=f[:], in1=Bf_bc[:], op=AO.add)
        nc.vector.tensor_scalar_max(f[:], f[:], 0.0)
        ftr_ps = ps_a.tile([32, 128], F32, tag="trp", name="ftr")
        nc.tensor.transpose(out=ftr_ps[:], in_=f[:], identity=ident[:])
        ftr = sb.tile([32, 128], F32, tag="ht", name="ftrs")
        nc.vector.tensor_copy(ftr[:], ftr_ps[:])
        op = ps_b.tile([2, 128], F32, tag="mmp", name="op")
        nc.tensor.matmul(out=op[:], lhsT=Wf2_sb[:], rhs=ftr[:],
                         start=True, stop=True)
        ot = sb.tile([2, 128], F32, tag="ot", name="ot")
        nc.scalar.activation(ot[:], op[:], AF.Tanh, bias=bf2_sb[:])
        nc.sync.dma_start(t_out[:, ts(t)], ot[:])


# ----------------------------------------------------------------------------
# Public entry point
# ----------------------------------------------------------------------------

_CACHE = {}


def _get_compiled(edge_index, x):
    key = hash((edge_index.tobytes(), x.shape))
    if key not in _CACHE:
        plan, streams = _preprocess(edge_index, x)
        nc = _build_program(plan)
        _CACHE.clear()
        _CACHE[key] = (nc, streams)
    return _CACHE[key]


def _in_maps(streams, kw):
    rep = dict(
        We=np.asarray(kw["We"], np.float32),
        W1=np.asarray(kw["W1"], np.float32),
        W2=np.asarray(kw["W2"], np.float32),
        W3=np.asarray(kw["W3"], np.float32),
        Wf1=np.asarray(kw["Wf1"], np.float32),
        Wf2=np.asarray(kw["Wf2"], np.float32),
        be_bc=np.tile(np.asarray(kw["be"], np.float32)[None, :], (128, 1)),
        g1=np.asarray(kw["g1"], np.float32)[None, :],
        bt1=np.asarray(kw["bt1"], np.float32)[None, :],
        g2=np.asarray(kw["g2"], np.float32)[None, :],
        bt2=np.asarray(kw["bt2"], np.float32)[None, :],
        g3=np.asarray(kw["g3"], np.float32)[None, :],
        bt3=np.asarray(kw["bt3"], np.float32)[None, :],
        gf=np.asarray(kw["gf"], np.float32)[None, :],
        btf=np.asarray(kw["btf"], np.float32)[None, :],
        bf2c=np.asarray(kw["bf2"], np.float32)[:, None],
    )
    return [dict(rep, **streams[c]) for c in range(NCORES)]


def run(trace=False, **kw):
    x = np.asarray(kw["x"], np.float32)
    edge_index = np.asarray(kw["edge_index"], np.int32)
    nc, streams = _get_compiled(edge_index, x)
    res = run_bass_kernel_spmd(nc, _in_maps(streams, kw),
                               core_ids=list(range(NCORES)), trace=trace)
    shards = [res.results[c]["out"][:, :NSH].T for c in range(NCORES)]
    out = np.ascontiguousarray(np.concatenate(shards, 0))
    return out, res


def kernel(**kw):
    out, _ = run(trace=False, **kw)
    return out

